# revision 4
# baseline (speedup 1.0000x reference)
"""Trainium2 Bass kernel v2 for nn_GAT_15994458210581.

BatchNorm(train) -> GATConv(8 heads, concat=False/mean, self-loops)
-> ELU -> global_mean_pool over 100 graphs.

v2 strategy (8 NeuronCores, SPMD):
- Phase A SHARDED: each core normalizes+projects only its N/8 node slice
  (x slice resident in SBUF, single read), BN stats via tiny AllReduce,
  then one AllGather replicates the bf16 node table [row: h(128 bf16) |
  a_src(8 f32) | pad] = 512B to every core.
- a_dst per dst-node kept resident in SBUF (copied out of the projection
  PSUM), so no per-block indirect gather.
- Phase B: per (dst-block, src-segment) group: one bf16 dma_gather;
  one-hots built cheaply (edge-major via tensor_scalar with per-partition
  scalar; dst-major via K=1 broadcast matmul + is_equal); leaky-relu/exp
  on the Activation engine; wide per-group vector ops; bf16 scatter
  matmuls accumulating per-block PSUM; per-graph pool via host-built
  one-hot matmul.
"""
import sys

sys.path.insert(0, "/opt/trn_rl_repo")
import numpy as np

EPS = 1e-5
NEG_SLOPE = 0.2
SENT = 200.0                       # sentinel dloc for pad edges (exact in bf16)


def _default_cfg():
    return dict(N=100000, F=128, H=8, C=16, G=100, NCORES=8, SEG=32768, MAXCH=8)


def _derive(cfg):
    d = dict(cfg)
    d["HC"] = d["H"] * d["C"]
    d["NPC"] = -(-d["N"] // d["NCORES"] // 128) * 128   # nodes per core (12544)
    d["B"] = d["NPC"] // 128                            # dst blocks per core (98)
    d["TROWS"] = d["NPC"] * d["NCORES"]                 # table rows (100352)
    d["NSEG"] = -(-d["TROWS"] // d["SEG"])              # src segments (4)
    d["ROWE"] = 256                                     # bf16 elements per row
    d["AOFF_F32"] = 64                                  # a_src at f32 slot 64 (byte 256)
    return d


def host_prep(cfg, edge_index, batch):
    c = cfg
    N, NC, NPC, B, SEG, NSEG = c["N"], c["NCORES"], c["NPC"], c["B"], c["SEG"], c["NSEG"]
    MAXCH, G = c["MAXCH"], c["G"]

    src = np.concatenate([np.asarray(edge_index[0]), np.arange(N)]).astype(np.int64)
    dst = np.concatenate([np.asarray(edge_index[1]), np.arange(N)]).astype(np.int64)
    batch = np.asarray(batch).astype(np.int64)

    core = np.minimum(dst // NPC, NC - 1)
    dloc = dst - core * NPC
    blk = dloc // 128
    seg = src // SEG

    cnt = np.zeros((NC, B, NSEG), np.int64)
    np.add.at(cnt, (core, blk, seg), 1)
    mx = cnt.max(axis=0)
    cnt16 = -(-mx // 16) * 16

    structure = []                       # (b, s, c16_piece)
    for b in range(B):
        for s in range(NSEG):
            left = int(cnt16[b, s])
            while left > 0:
                piece = min(left, MAXCH * 128)
                structure.append((b, s, piece))
                left -= piece

    coloff, choff = [], []
    o_c = o_ch = 0
    for (b, s, c16) in structure:
        coloff.append(o_c); choff.append(o_ch)
        o_c += c16 // 16
        o_ch += -(-c16 // 128)
    TOTCOLS, TOTCH = max(o_c, 1), max(o_ch, 1)

    # flat drel stream (one value per padded edge slot, chunk-padded to 128),
    # staged to SBUF in batches of STAGE_G groups
    STAGE_G = 8
    rowoff, batch_of, stage_start = [], [], []
    o_r = 0
    for gi, (b, s, c16) in enumerate(structure):
        if gi % STAGE_G == 0:
            stage_start.append([o_r, 0])
        batch_of.append(len(stage_start) - 1)
        rowoff.append(o_r)
        o_r += -(-c16 // 128) * 128
        stage_start[-1][1] = o_r - stage_start[-1][0]
    TOTROW = max(o_r, 1)
    SCAP = max((s[1] for s in stage_start), default=1)

    order = np.lexsort((seg, blk, core))
    src_s, blk_s, seg_s, dloc_s = src[order], blk[order], seg[order], dloc[order]
    core_s = core[order]
    key = (core_s * B + blk_s) * NSEG + seg_s
    kk = np.arange(NC * B * NSEG)
    starts = np.searchsorted(key, kk)
    ends = np.searchsorted(key, kk + 1)

    idx16 = np.zeros((NC, 128, TOTCOLS), np.int16)
    drc = np.full((NC, 128, TOTCH), SENT, np.float32)
    drel_row = np.full((NC, 1, TOTROW), SENT, np.float32)

    consumed = {}
    for gi, (b, s, c16) in enumerate(structure):
        nch = -(-c16 // 128)
        for m in range(NC):
            k = (m * B + b) * NSEG + s
            st, en = starts[k], ends[k]
            already = consumed.get((m, b, s), 0)
            take = max(0, min(en - st - already, c16))
            lo = st + already
            consumed[(m, b, s)] = already + take
            idxs = np.zeros(c16, np.int16)
            drels = np.full(nch * 128, SENT, np.float32)
            if take > 0:
                idxs[:take] = (src_s[lo:lo + take] - s * SEG).astype(np.int16)
                drels[:take] = (dloc_s[lo:lo + take] - b * 128).astype(np.float32)
            wrapped = idxs.reshape(c16 // 16, 16).T
            idx16[m, :, coloff[gi]:coloff[gi] + c16 // 16] = np.tile(wrapped, (8, 1))
            dr = drels.reshape(nch, 128)
            drc[m, :, choff[gi]:choff[gi] + nch] = dr.T
            drel_row[m, 0, rowoff[gi]:rowoff[gi] + nch * 128] = drels

    # host-built per-graph one-hot (pool matmul lhsT), [128, B*128] per core:
    # onG[d_part, b*128 + gcol] = 1 if node (b*128+d_part) of this core is in
    # graph g0[m]+gcol else 0; pad nodes get all-zero columns.
    onG = np.zeros((NC, 128, B * 128), np.float32)
    g0 = np.zeros(NC, np.int64)
    for m in range(NC):
        lo = m * NPC
        hi = min(lo + NPC, N)
        n = hi - lo
        g0[m] = batch[min(lo, N - 1)]
        if n > 0:
            grel = (batch[lo:hi] - g0[m]).astype(np.int64)   # 0..127
            dl = np.arange(n)
            onG[m, dl % 128, (dl // 128) * 128 + grel] = 1.0

    return dict(structure=structure, coloff=coloff, choff=choff,
                rowoff=rowoff, batch_of=batch_of, stage_start=stage_start,
                STAGE_G=STAGE_G, TOTCOLS=TOTCOLS, TOTCH=TOTCH, TOTROW=TOTROW,
                SCAP=SCAP, idx16=idx16, drc=drc, drel_row=drel_row,
                onG=onG, g0=g0)


def build_nc(cfg, prep, num_devices, variant='full'):
    import concourse.bass as bass
    import concourse.bacc as bacc
    import concourse.mybir as mybir
    from concourse.tile import TileContext
    from contextlib import ExitStack

    c = cfg
    f32 = mybir.dt.float32
    bf16 = mybir.dt.bfloat16
    i16 = mybir.dt.int16
    Alu = mybir.AluOpType
    Act = mybir.ActivationFunctionType
    N, F, HC, H, Cc, G = c["N"], c["F"], c["HC"], c["H"], c["C"], c["G"]
    NPC, B, SEG, NSEG, TROWS = c["NPC"], c["B"], c["SEG"], c["NSEG"], c["TROWS"]
    MAXCH, ROWE = c["MAXCH"], c["ROWE"]
    structure, coloff, choff = prep["structure"], prep["coloff"], prep["choff"]
    rowoff, batch_of, stage_start = prep["rowoff"], prep["batch_of"], prep["stage_start"]
    STAGE_G = prep["STAGE_G"]
    TOTCOLS, TOTCH, TOTROW, SCAP = (prep["TOTCOLS"], prep["TOTCH"],
                                    prep["TOTROW"], prep["SCAP"])
    NT = B                      # node tiles per core slice
    groups = [list(range(num_devices))]

    nc = bacc.Bacc("TRN2", target_bir_lowering=False, debug=False,
                   num_devices=num_devices)
    xs_d = nc.dram_tensor("xs", [NPC, F], f32, kind="ExternalInput")
    W_d = nc.dram_tensor("W", [F, HC], f32, kind="ExternalInput")
    gamma_d = nc.dram_tensor("gamma", [F, 1], f32, kind="ExternalInput")
    beta_d = nc.dram_tensor("beta", [F, 1], f32, kind="ExternalInput")
    attb_d = nc.dram_tensor("attboth", [HC, 16], bf16, kind="ExternalInput")
    biasm_d = nc.dram_tensor("bias_mat", [128, Cc], f32, kind="ExternalInput")
    identf_d = nc.dram_tensor("ident_f", [128, 128], f32, kind="ExternalInput")
    identb_d = nc.dram_tensor("ident_b", [128, 128], bf16, kind="ExternalInput")
    iotamb_d = nc.dram_tensor("iotam_b", [128, 128], bf16, kind="ExternalInput")
    iotacf_d = nc.dram_tensor("iotac_f", [128, 1], f32, kind="ExternalInput")
    onesb_d = nc.dram_tensor("ones_b", [1, 128], bf16, kind="ExternalInput")
    onG_d = nc.dram_tensor("onG", [128, B * 128], bf16, kind="ExternalInput")
    idx_d = nc.dram_tensor("idx16", [128, TOTCOLS], i16, kind="ExternalInput")
    drc_d = nc.dram_tensor("drc", [128, TOTCH], f32, kind="ExternalInput")
    drel_d = nc.dram_tensor("drel_row", [1, TOTROW], bf16, kind="ExternalInput")
    out_d = nc.dram_tensor("pool_out", [128, Cc], f32, kind="ExternalOutput")

    stats_d = nc.dram_tensor("stats_loc", [F, 2], f32)
    stats_sh = nc.dram_tensor("stats_sh", [F, 2], f32, addr_space="Shared")
    tslice_d = nc.dram_tensor("tslice", [NPC, ROWE], bf16)
    table_sh = nc.dram_tensor("table_sh", [TROWS, ROWE], bf16, addr_space="Shared")

    seg_lo = [s * SEG for s in range(NSEG)]
    segrows = [min(SEG, TROWS - s * SEG) for s in range(NSEG)]

    with TileContext(nc) as tc, ExitStack() as ctx:
        cp = ctx.enter_context(tc.tile_pool(name="consts", bufs=1))

        def cload(name, dram, shape, dt=f32):
            t = cp.tile(shape, dt, tag=name)
            nc.sync.dma_start(out=t[:], in_=dram[:, :])
            return t

        W_t = cload("W", W_d, [F, HC])
        gamma_t = cload("gam", gamma_d, [F, 1])
        beta_t = cload("bet", beta_d, [F, 1])
        attb_t = cload("attb", attb_d, [HC, 16], bf16)
        biasm_t = cload("biasm", biasm_d, [128, Cc])
        identf_t = cload("identf", identf_d, [128, 128])
        identb_t = cload("identb", identb_d, [128, 128], bf16)
        iotamb_t = cload("iotamb", iotamb_d, [128, 128], bf16)
        iotacf_t = cload("iotacf", iotacf_d, [128, 1])
        onesb_t = cload("onesb", onesb_d, [1, 128], bf16)
        onG_t = cload("onG", onG_d, [128, B * 128], bf16)
        idx_t = cload("idx", idx_d, [128, TOTCOLS], i16)
        drc_t = cload("drc", drc_d, [128, TOTCH])
        ones_t = cp.tile([128, 1], f32, tag="ones")
        nc.vector.memset(ones_t[:], 1.0)
        scale_t = cp.tile([F, 1], f32, tag="scl")
        shift_t = cp.tile([F, 1], f32, tag="shf")
        Wp_t = cp.tile([F, HC], f32, tag="Wp")
        Wpb_t = cp.tile([F, HC], bf16, tag="Wpb")
        c0_t = cp.tile([HC, 1], f32, tag="c0")
        adw_t = cp.tile([128, B * H], bf16, tag="adw")
        xpool = tc.tile_pool(name="xp", bufs=1)
        xp = xpool.__enter__()
        xres = xp.tile([128, NT, F], f32, tag="xres")

        # ---------------- Phase A: load x slice (resident) ----------------
        XSP = 4           # node tiles per load/store DMA
        for k0 in range(0, NT, XSP):
            kk = min(XSP, NT - k0)
            nc.sync.dma_start(
                out=xres[:, k0:k0 + kk, :],
                in_=xs_d[k0 * 128:(k0 + kk) * 128, :].rearrange(
                    "(a p) f -> p a f", p=128))

        # ---------------- pass 1: BN stats + AllReduce ----------------
        with tc.tile_pool(name="p1", bufs=3) as p1, \
             tc.tile_pool(name="p1ps", bufs=1, space="PSUM") as p1ps:
            statx = p1ps.tile([F, 1], f32, tag="sx")
            statx2 = p1ps.tile([F, 1], f32, tag="sx2")
            for t in range(NT):
                sq = p1.tile([128, F], f32, tag="sq")
                nc.vector.tensor_tensor(out=sq[:], in0=xres[:, t, :],
                                        in1=xres[:, t, :], op=Alu.mult)
                nc.tensor.matmul(out=statx[:], lhsT=xres[:, t, :], rhs=ones_t[:],
                                 start=(t == 0), stop=(t == NT - 1))
                nc.tensor.matmul(out=statx2[:], lhsT=sq[:], rhs=ones_t[:],
                                 start=(t == 0), stop=(t == NT - 1))
            sloc = p1.tile([F, 2], f32, tag="sloc")
            nc.vector.tensor_copy(out=sloc[:, 0:1], in_=statx[:])
            nc.vector.tensor_copy(out=sloc[:, 1:2], in_=statx2[:])
            nc.sync.dma_start(out=stats_d[:, :], in_=sloc[:])
            nc.gpsimd.collective_compute(
                kind="AllReduce", op=Alu.add, replica_groups=groups,
                ins=[stats_d[:, :]], outs=[stats_sh[:, :]])
            tc.strict_bb_all_engine_barrier()
            sglob = p1.tile([F, 2], f32, tag="sglob")
            nc.sync.dma_start(out=sglob[:], in_=stats_sh[:, :])
            mean_t = p1.tile([F, 1], f32, tag="mean")
            tmp = p1.tile([F, 1], f32, tag="tmp")
            tmp2 = p1.tile([F, 1], f32, tag="tmp2")
            nc.vector.tensor_scalar_mul(out=mean_t[:], in0=sglob[:, 0:1], scalar1=1.0 / N)
            nc.vector.tensor_scalar_mul(out=tmp[:], in0=sglob[:, 1:2], scalar1=1.0 / N)
            nc.vector.tensor_tensor(out=tmp2[:], in0=mean_t[:], in1=mean_t[:], op=Alu.mult)
            nc.vector.tensor_tensor(out=tmp[:], in0=tmp[:], in1=tmp2[:], op=Alu.subtract)
            nc.vector.tensor_scalar_add(out=tmp[:], in0=tmp[:], scalar1=EPS)
            nc.scalar.activation(out=tmp[:], in_=tmp[:], func=Act.Sqrt)
            nc.vector.reciprocal(out=tmp2[:], in_=tmp[:])
            nc.vector.tensor_tensor(out=scale_t[:], in0=tmp2[:], in1=gamma_t[:], op=Alu.mult)
            nc.vector.tensor_tensor(out=tmp[:], in0=mean_t[:], in1=scale_t[:], op=Alu.mult)
            nc.vector.tensor_tensor(out=shift_t[:], in0=beta_t[:], in1=tmp[:], op=Alu.subtract)
            nc.vector.tensor_scalar_mul(out=Wp_t[:], in0=W_t[:], scalar1=scale_t[:, 0:1])
            nc.scalar.activation(out=Wpb_t[:], in_=Wp_t[:], func=Act.Identity)
            c0ps = p1ps.tile([HC, 1], f32, tag="c0p")
            nc.tensor.matmul(out=c0ps[:], lhsT=W_t[:], rhs=shift_t[:], start=True, stop=True)
            nc.vector.tensor_copy(out=c0_t[:], in_=c0ps[:])

        # ---------------- pass 2: node table slice ----------------
        with tc.tile_pool(name="p2", bufs=3) as p2, \
             tc.tile_pool(name="p2r", bufs=2) as p2r, \
             tc.tile_pool(name="p2ps", bufs=2, space="PSUM") as p2ps:
            row4 = None
            for t in range(NT if variant != 'a1' else 0):
                j = t % XSP
                if j == 0:
                    row4 = p2r.tile([128, XSP, ROWE], bf16, tag="row4")
                    if t < 2 * XSP:
                        # first use of each ring buffer: clear pad region once
                        nc.vector.memset(row4[:], 0.0)
                xTp = p2ps.tile([F, 128], f32, tag="xT")
                nc.tensor.transpose(out=xTp[:], in_=xres[:, t, :], identity=identf_t[:])
                xTb = p2.tile([F, 128], bf16, tag="xTb")
                nc.scalar.activation(out=xTb[:], in_=xTp[:], func=Act.Identity)
                hTp = p2ps.tile([HC, 128], f32, tag="hT")
                nc.tensor.matmul(out=hTp[:], lhsT=Wpb_t[:], rhs=xTb[:], start=True, stop=True)
                hTb = p2.tile([HC, 128], bf16, tag="hTb")
                nc.scalar.activation(out=hTb[:], in_=hTp[:], func=Act.Identity, bias=c0_t[:, 0:1])
                ap_ = p2ps.tile([128, 16], f32, tag="a")
                nc.tensor.matmul(out=ap_[:], lhsT=hTb[:], rhs=attb_t[:], start=True, stop=True)
                hpp = p2ps.tile([128, HC], bf16, tag="hp")
                nc.tensor.transpose(out=hpp[:], in_=hTb[:], identity=identb_t[:])
                nc.scalar.activation(out=row4[:, j, 0:HC], in_=hpp[:], func=Act.Identity)
                nc.vector.tensor_copy(
                    out=row4[:].bitcast(f32)[:, j, 64:72], in_=ap_[:, 0:8])
                nc.scalar.activation(out=adw_t[:, t * H:(t + 1) * H],
                                     in_=ap_[:, 8:16], func=Act.Identity)
                if j == XSP - 1 or t == NT - 1:
                    kk = j + 1
                    base = (t - j) * 128
                    nc.sync.dma_start(
                        out=tslice_d[base:base + kk * 128, :].rearrange(
                            "(a p) c -> p a c", p=128),
                        in_=row4[:, 0:kk, :])

        xpool.__exit__(None, None, None)   # free the x slice SBUF for phase B

        if variant not in ('a1', 'a2'):
            nc.gpsimd.collective_compute(
                kind="AllGather", op=Alu.bypass, replica_groups=groups,
                ins=[tslice_d[:, :]], outs=[table_sh[:, :]])
            tc.strict_bb_all_engine_barrier()

        # ---------------- Phase B ----------------
        GBUFS = 4
        gb = ctx.enter_context(tc.tile_pool(name="g", bufs=GBUFS))
        stg = ctx.enter_context(tc.tile_pool(name="stg", bufs=2))
        onep = ctx.enter_context(tc.tile_pool(name="onehots", bufs=4))
        wpool = ctx.enter_context(tc.tile_pool(name="wts", bufs=4))
        rp = ctx.enter_context(tc.tile_pool(name="rhs", bufs=4))
        pp = ctx.enter_context(tc.tile_pool(name="post", bufs=2))
        ups = ctx.enter_context(tc.tile_pool(name="ups", bufs=2, space="PSUM"))
        aps = ctx.enter_context(tc.tile_pool(name="aps", bufs=2, space="PSUM"))
        dbp = ctx.enter_context(tc.tile_pool(name="dbp", bufs=2, space="PSUM"))
        gps = ctx.enter_context(tc.tile_pool(name="gps", bufs=1, space="PSUM"))

        pool_ps = gps.tile([128, Cc], f32, tag="pool")

        cur_batch = [-1]
        cur_stage = [None]
        cur_soff = [0]
        by_block = [[] for _ in range(B)]
        for gi, (b, s, c16) in enumerate(structure):
            by_block[b].append((gi, s, c16))

        for b in range(B if variant in ('full',) else 0):
            u_ps = ups.tile([128, HC + H], f32, tag="u")
            nch_b = sum(-(-c16 // 128) for (_, _, c16) in by_block[b])
            ci = 0
            for (gi, s, c16) in by_block[b]:
                nch = -(-c16 // 128)
                L = nch * 128
                g = gb.tile([128, MAXCH, ROWE], bf16, tag="g")
                if gi < GBUFS:
                    # first use of each ring buffer: clear uninitialized SBUF so
                    # stale-NaN bit patterns can't poison pad-edge lanes; later
                    # groups only ever re-read finite gathered rows
                    nc.vector.memset(g[:], 0.0)
                nc.gpsimd.dma_gather(
                    out_ap=g[:, 0:nch, :],
                    in_ap=table_sh[seg_lo[s]:seg_lo[s] + segrows[s], :],
                    idxs_ap=idx_t[:, coloff[gi]:coloff[gi] + c16 // 16],
                    num_idxs=c16, num_idxs_reg=c16, elem_size=ROWE,
                    single_packet=False)
                if batch_of[gi] != cur_batch[0]:
                    cur_batch[0] = batch_of[gi]
                    soff, slen = stage_start[cur_batch[0]]
                    st_t = stg.tile([1, SCAP], bf16, tag="stage")
                    nc.sync.dma_start(out=st_t[0:1, 0:slen],
                                      in_=drel_d[0:1, soff:soff + slen])
                    cur_stage[0] = st_t
                    cur_soff[0] = soff
                st_t = cur_stage[0]
                roff = rowoff[gi] - cur_soff[0]

                onN = onep.tile([128, MAXCH * 128], bf16, tag="onN")
                for k in range(0, L, 512):
                    Lk = min(512, L - k)
                    dbc = dbp.tile([128, 512], f32, tag="dbc")
                    nc.tensor.matmul(out=dbc[:, 0:Lk], lhsT=onesb_t[:],
                                     rhs=st_t[0:1, roff + k:roff + k + Lk],
                                     start=True, stop=True)
                    # keep the compare off Pool: gather desc-gen queues behind
                    # it in Pool's in-order SEQ and the whole pipeline stalls
                    eng = nc.vector
                    eng.tensor_tensor(
                        out=onN[:, k:k + Lk],
                        in0=iotacf_t[:, 0:1].to_broadcast([128, Lk]),
                        in1=dbc[:, 0:Lk], op=Alu.is_equal)
                onE = onep.tile([128, MAXCH * 128], bf16, tag="onE")
                ae_ps = aps.tile([128, MAXCH * H], f32, tag="ae")
                for cch in range(nch):
                    nc.vector.tensor_scalar(
                        out=onE[:, cch * 128:(cch + 1) * 128], in0=iotamb_t[:],
                        scalar1=drc_t[:, choff[gi] + cch:choff[gi] + cch + 1],
                        scalar2=None, op0=Alu.is_equal)
                    nc.tensor.matmul(out=ae_ps[:, cch * H:(cch + 1) * H],
                                     lhsT=onN[:, cch * 128:(cch + 1) * 128],
                                     rhs=adw_t[:, b * H:(b + 1) * H],
                                     start=True, stop=True)
                egrp = wpool.tile([128, MAXCH * H], f32, tag="egrp")
                nc.vector.tensor_tensor(
                    out=egrp[:, 0:nch * H].rearrange("p (c h) -> p c h", h=H),
                    in0=g[:].bitcast(f32)[:, 0:nch, 64:72],
                    in1=ae_ps[:, 0:nch * H].rearrange("p (c h) -> p c h", h=H),
                    op=Alu.add)
                t1 = wpool.tile([128, MAXCH * H], f32, tag="t1")
                nc.vector.scalar_tensor_tensor(
                    out=t1[:, 0:nch * H], in0=egrp[:, 0:nch * H], scalar=NEG_SLOPE,
                    in1=egrp[:, 0:nch * H], op0=Alu.mult, op1=Alu.max)
                # expand exp(logit) to all 16 channels on ACT (same Exp table,
                # broadcast input) so the weight-multiply runs packed-2x on DVE
                wtx = wpool.tile([128, MAXCH, H, Cc], bf16, tag="wtx")
                nc.scalar.activation(
                    out=wtx[:, 0:nch, :, :],
                    in_=t1[:, 0:nch * H].rearrange("p (c h one) -> p c h one", h=H,
                                                   one=1).to_broadcast([128, nch, H, Cc]),
                    func=Act.Exp)
                rhs_t = rp.tile([128, MAXCH, HC + H], bf16, tag="rhs")
                nc.vector.tensor_tensor(
                    out=rhs_t[:, 0:nch, 0:HC].rearrange("p c (h c2) -> p c h c2", h=H),
                    in0=g[:, 0:nch, 0:HC].rearrange("p c (h c2) -> p c h c2", h=H),
                    in1=wtx[:, 0:nch, :, :],
                    op=Alu.mult)
                nc.vector.tensor_copy(out=rhs_t[:, 0:nch, HC:HC + H],
                                      in_=wtx[:, 0:nch, :, 0].rearrange("p c h -> p (c h)"))
                for cch in range(nch):
                    nc.tensor.matmul(out=u_ps[:], lhsT=onE[:, cch * 128:(cch + 1) * 128],
                                     rhs=rhs_t[:, cch, :],
                                     start=(ci == 0), stop=(ci == nch_b - 1))
                    ci += 1
            # ---- postprocess block ----
            s_sb = pp.tile([128, H], f32, tag="s")
            nc.vector.tensor_scalar_add(out=s_sb[:], in0=u_ps[:, HC:HC + H], scalar1=1e-30)
            rs = pp.tile([128, H], f32, tag="rs")
            nc.vector.reciprocal(out=rs[:], in_=s_sb[:])
            prod = pp.tile([128, HC], f32, tag="prod")
            nc.vector.tensor_tensor(
                out=prod[:].rearrange("p (h c2) -> p h c2", h=H),
                in0=u_ps[:, 0:HC].rearrange("p (h c2) -> p h c2", h=H),
                in1=rs[:].rearrange("p (h one) -> p h one", h=H
                                    ).to_broadcast([128, H, Cc]),
                op=Alu.mult)
            o16 = pp.tile([128, Cc], f32, tag="o16")
            nc.vector.tensor_reduce(out=o16[:], in_=prod[:].rearrange("p (h c2) -> p c2 h", h=H),
                                    axis=mybir.AxisListType.X, op=Alu.add)
            o16b = pp.tile([128, Cc], f32, tag="o16b")
            nc.vector.scalar_tensor_tensor(out=o16b[:], in0=o16[:], scalar=1.0 / H,
                                           in1=biasm_t[:], op0=Alu.mult, op1=Alu.add)
            m0 = pp.tile([128, Cc], f32, tag="m0")
            nc.vector.tensor_scalar(out=m0[:], in0=o16b[:], scalar1=0.0, scalar2=None,
                                    op0=Alu.min)
            em = pp.tile([128, Cc], f32, tag="em")
            nc.scalar.activation(out=em[:], in_=m0[:], func=Act.Exp)
            r0 = pp.tile([128, Cc], f32, tag="r0")
            nc.vector.scalar_tensor_tensor(out=r0[:], in0=m0[:], scalar=-1.0,
                                           in1=o16b[:], op0=Alu.mult, op1=Alu.add)
            onode = pp.tile([128, Cc], bf16, tag="onode")
            nc.vector.scalar_tensor_tensor(out=onode[:], in0=em[:], scalar=-1.0,
                                           in1=r0[:], op0=Alu.add, op1=Alu.add)
            nc.tensor.matmul(out=pool_ps[:], lhsT=onG_t[:, b * 128:(b + 1) * 128],
                             rhs=onode[:], start=(b == 0), stop=(b == B - 1))

        if variant in ('full',):
            outp_t = pp.tile([128, Cc], f32, tag="out")
            nc.vector.tensor_copy(out=outp_t[:], in_=pool_ps[:])
            nc.sync.dma_start(out=out_d[:, :], in_=outp_t[:])
        else:
            outp_t = pp.tile([128, Cc], f32, tag="out")
            nc.vector.memset(outp_t[:], 0.0)
            nc.sync.dma_start(out=out_d[:, :], in_=outp_t[:])

    nc.compile()
    return nc


def _np_f32(a):
    return np.ascontiguousarray(np.asarray(a), dtype=np.float32)


def make_in_maps(cfg, prep, inputs):
    c = cfg
    F, H, Cc, HC, NC, NPC, B, N = (c["F"], c["H"], c["C"], c["HC"], c["NCORES"],
                                   c["NPC"], c["B"], c["N"])
    x = _np_f32(inputs["x"])
    W = _np_f32(inputs["W"])
    gamma = _np_f32(inputs["bn_gamma"]).reshape(F, 1)
    beta = _np_f32(inputs["bn_beta"]).reshape(F, 1)
    att_src = _np_f32(inputs["att_src"])
    att_dst = _np_f32(inputs["att_dst"])
    bias = _np_f32(inputs["bias"]).reshape(1, Cc)

    def bf(a):
        import jax.numpy as jnp
        return np.asarray(jnp.asarray(a, dtype=jnp.bfloat16))

    attboth = np.zeros((HC, 16), np.float32)
    for h in range(H):
        attboth[h * Cc:(h + 1) * Cc, h] = att_src[h]
        attboth[h * Cc:(h + 1) * Cc, 8 + h] = att_dst[h]

    xpad = np.zeros((NPC * NC, F), np.float32)
    xpad[:N] = x

    iotam = np.tile(np.arange(128, dtype=np.float32), (128, 1))
    shared = dict(
        W=W, gamma=gamma, beta=beta,
        attboth=bf(attboth),
        bias_mat=np.tile(bias, (128, 1)),
        ident_f=np.eye(128, dtype=np.float32),
        ident_b=bf(np.eye(128)),
        iotam_b=bf(iotam),
        iotac_f=np.arange(128, dtype=np.float32).reshape(128, 1),
        ones_b=bf(np.ones((1, 128))),
    )
    in_maps = []
    for m in range(NC):
        im = dict(shared)
        im["xs"] = xpad[m * NPC:(m + 1) * NPC]
        im["idx16"] = prep["idx16"][m]
        im["drc"] = prep["drc"][m]
        im["drel_row"] = bf(prep["drel_row"][m])
        im["onG"] = bf(prep["onG"][m])
        in_maps.append(im)
    return in_maps


def unshard(cfg, prep, results):
    c = cfg
    G, Cc, NC = c["G"], c["C"], c["NCORES"]
    batchcnt = prep["graph_counts"]
    out = np.zeros((G, Cc), np.float64)
    for m in range(NC):
        pool_m = results[m]["pool_out"]
        g0 = int(prep["g0"][m])
        hi = min(128, G - g0)
        out[g0:g0 + hi] += pool_m[:hi]
    out = out / np.maximum(batchcnt, 1.0)[:, None]
    return out.astype(np.float32)


_CACHE = {}
LAST = {}


def kernel(**inputs):
    from concourse.bass_utils import run_bass_kernel_spmd

    cfg = _derive(_default_cfg())
    batch = np.asarray(inputs["batch"]).astype(np.int64)
    prep = host_prep(cfg, inputs["edge_index"], batch)
    prep["graph_counts"] = np.bincount(batch, minlength=cfg["G"]).astype(np.float64)
    key = "full"
    if key not in _CACHE:
        _CACHE[key] = build_nc(cfg, prep, cfg["NCORES"])
    nc = _CACHE[key]
    in_maps = make_in_maps(cfg, prep, inputs)
    res = run_bass_kernel_spmd(nc, in_maps, list(range(cfg["NCORES"])))
    LAST["res"] = res
    return unshard(cfg, prep, res.results)


# revision 6
# speedup vs baseline: 1.4417x; 1.4417x over previous
"""Trainium2 Bass kernel v2 for nn_GAT_15994458210581.

BatchNorm(train) -> GATConv(8 heads, concat=False/mean, self-loops)
-> ELU -> global_mean_pool over 100 graphs.

v2 strategy (8 NeuronCores, SPMD):
- Phase A SHARDED: each core normalizes+projects only its N/8 node slice
  (x slice resident in SBUF, single read), BN stats via tiny AllReduce,
  then one AllGather replicates the bf16 node table [row: h(128 bf16) |
  a_src(8 f32) | pad] = 512B to every core.
- a_dst per dst-node kept resident in SBUF (copied out of the projection
  PSUM), so no per-block indirect gather.
- Phase B: per (dst-block, src-segment) group: one bf16 dma_gather;
  one-hots built cheaply (edge-major via tensor_scalar with per-partition
  scalar; dst-major via K=1 broadcast matmul + is_equal); leaky-relu/exp
  on the Activation engine; wide per-group vector ops; bf16 scatter
  matmuls accumulating per-block PSUM; per-graph pool via host-built
  one-hot matmul.
"""
import sys

sys.path.insert(0, "/opt/trn_rl_repo")
import numpy as np

EPS = 1e-5
NEG_SLOPE = 0.2
SENT = 200.0                       # sentinel dloc for pad edges (exact in bf16)


def _default_cfg():
    return dict(N=100000, F=128, H=8, C=16, G=100, NCORES=8, SEG=32768, MAXCH=8)


def _derive(cfg):
    d = dict(cfg)
    d["HC"] = d["H"] * d["C"]
    d["NPC"] = -(-d["N"] // d["NCORES"] // 128) * 128   # nodes per core (12544)
    d["B"] = d["NPC"] // 128                            # dst blocks per core (98)
    d["TROWS"] = d["NPC"] * d["NCORES"]                 # table rows (100352)
    d["NSEG"] = -(-d["TROWS"] // d["SEG"])              # src segments (4)
    d["ROWE"] = 256                                     # bf16 elements per row
    d["AOFF_F32"] = 64                                  # a_src at f32 slot 64 (byte 256)
    return d


def host_prep(cfg, edge_index, batch):
    c = cfg
    N, NC, NPC, B, SEG, NSEG = c["N"], c["NCORES"], c["NPC"], c["B"], c["SEG"], c["NSEG"]
    MAXCH, G = c["MAXCH"], c["G"]

    src = np.concatenate([np.asarray(edge_index[0]), np.arange(N)]).astype(np.int64)
    dst = np.concatenate([np.asarray(edge_index[1]), np.arange(N)]).astype(np.int64)
    batch = np.asarray(batch).astype(np.int64)

    core = np.minimum(dst // NPC, NC - 1)
    dloc = dst - core * NPC
    blk = dloc // 128
    seg = src // SEG

    cnt = np.zeros((NC, B, NSEG), np.int64)
    np.add.at(cnt, (core, blk, seg), 1)
    mx = cnt.max(axis=0)
    cnt16 = -(-mx // 16) * 16

    structure = []                       # (b, s, c16_piece)
    for b in range(B):
        for s in range(NSEG):
            left = int(cnt16[b, s])
            while left > 0:
                piece = min(left, MAXCH * 128)
                structure.append((b, s, piece))
                left -= piece

    coloff, choff = [], []
    o_c = o_ch = 0
    for (b, s, c16) in structure:
        coloff.append(o_c); choff.append(o_ch)
        o_c += c16 // 16
        o_ch += -(-c16 // 128)
    TOTCOLS, TOTCH = max(o_c, 1), max(o_ch, 1)

    # flat drel stream (one value per padded edge slot, chunk-padded to 128),
    # staged to SBUF in batches of STAGE_G groups
    STAGE_G = 8
    rowoff, batch_of, stage_start = [], [], []
    o_r = 0
    for gi, (b, s, c16) in enumerate(structure):
        if gi % STAGE_G == 0:
            stage_start.append([o_r, 0])
        batch_of.append(len(stage_start) - 1)
        rowoff.append(o_r)
        o_r += -(-c16 // 128) * 128
        stage_start[-1][1] = o_r - stage_start[-1][0]
    TOTROW = max(o_r, 1)
    SCAP = max((s[1] for s in stage_start), default=1)

    order = np.lexsort((seg, blk, core))
    src_s, blk_s, seg_s, dloc_s = src[order], blk[order], seg[order], dloc[order]
    core_s = core[order]
    key = (core_s * B + blk_s) * NSEG + seg_s
    kk = np.arange(NC * B * NSEG)
    starts = np.searchsorted(key, kk)
    ends = np.searchsorted(key, kk + 1)

    idx16 = np.zeros((NC, 128, TOTCOLS), np.int16)
    drc = np.full((NC, 128, TOTCH), SENT, np.float32)
    drel_row = np.full((NC, 1, TOTROW), SENT, np.float32)

    consumed = {}
    for gi, (b, s, c16) in enumerate(structure):
        nch = -(-c16 // 128)
        for m in range(NC):
            k = (m * B + b) * NSEG + s
            st, en = starts[k], ends[k]
            already = consumed.get((m, b, s), 0)
            take = max(0, min(en - st - already, c16))
            lo = st + already
            consumed[(m, b, s)] = already + take
            idxs = np.zeros(c16, np.int16)
            drels = np.full(nch * 128, SENT, np.float32)
            if take > 0:
                idxs[:take] = (src_s[lo:lo + take] - s * SEG).astype(np.int16)
                drels[:take] = (dloc_s[lo:lo + take] - b * 128).astype(np.float32)
            wrapped = idxs.reshape(c16 // 16, 16).T
            idx16[m, :, coloff[gi]:coloff[gi] + c16 // 16] = np.tile(wrapped, (8, 1))
            dr = drels.reshape(nch, 128)
            drc[m, :, choff[gi]:choff[gi] + nch] = dr.T
            drel_row[m, 0, rowoff[gi]:rowoff[gi] + nch * 128] = drels

    # host-built per-graph one-hot (pool matmul lhsT), [128, B*128] per core:
    # onG[d_part, b*128 + gcol] = 1 if node (b*128+d_part) of this core is in
    # graph g0[m]+gcol else 0; pad nodes get all-zero columns.
    onG = np.zeros((NC, 128, B * 128), np.float32)
    g0 = np.zeros(NC, np.int64)
    for m in range(NC):
        lo = m * NPC
        hi = min(lo + NPC, N)
        n = hi - lo
        g0[m] = batch[min(lo, N - 1)]
        if n > 0:
            grel = (batch[lo:hi] - g0[m]).astype(np.int64)   # 0..127
            dl = np.arange(n)
            onG[m, dl % 128, (dl // 128) * 128 + grel] = 1.0

    return dict(structure=structure, coloff=coloff, choff=choff,
                rowoff=rowoff, batch_of=batch_of, stage_start=stage_start,
                STAGE_G=STAGE_G, TOTCOLS=TOTCOLS, TOTCH=TOTCH, TOTROW=TOTROW,
                SCAP=SCAP, idx16=idx16, drc=drc, drel_row=drel_row,
                onG=onG, g0=g0)


def build_nc(cfg, prep, num_devices, variant='full'):
    import concourse.bass as bass
    import concourse.bacc as bacc
    import concourse.mybir as mybir
    from concourse.tile import TileContext
    from contextlib import ExitStack

    c = cfg
    f32 = mybir.dt.float32
    bf16 = mybir.dt.bfloat16
    i16 = mybir.dt.int16
    Alu = mybir.AluOpType
    Act = mybir.ActivationFunctionType
    N, F, HC, H, Cc, G = c["N"], c["F"], c["HC"], c["H"], c["C"], c["G"]
    NPC, B, SEG, NSEG, TROWS = c["NPC"], c["B"], c["SEG"], c["NSEG"], c["TROWS"]
    MAXCH, ROWE = c["MAXCH"], c["ROWE"]
    structure, coloff, choff = prep["structure"], prep["coloff"], prep["choff"]
    rowoff, batch_of, stage_start = prep["rowoff"], prep["batch_of"], prep["stage_start"]
    STAGE_G = prep["STAGE_G"]
    TOTCOLS, TOTCH, TOTROW, SCAP = (prep["TOTCOLS"], prep["TOTCH"],
                                    prep["TOTROW"], prep["SCAP"])
    NT = B                      # node tiles per core slice
    groups = [list(range(num_devices))]

    nc = bacc.Bacc("TRN2", target_bir_lowering=False, debug=False,
                   num_devices=num_devices)
    xs_d = nc.dram_tensor("xs", [NPC, F], f32, kind="ExternalInput")
    W_d = nc.dram_tensor("W", [F, HC], f32, kind="ExternalInput")
    gamma_d = nc.dram_tensor("gamma", [F, 1], f32, kind="ExternalInput")
    beta_d = nc.dram_tensor("beta", [F, 1], f32, kind="ExternalInput")
    attb_d = nc.dram_tensor("attboth", [HC, 16], bf16, kind="ExternalInput")
    biasm_d = nc.dram_tensor("bias_mat", [128, Cc], f32, kind="ExternalInput")
    identf_d = nc.dram_tensor("ident_f", [128, 128], f32, kind="ExternalInput")
    identb_d = nc.dram_tensor("ident_b", [128, 128], bf16, kind="ExternalInput")
    iotamb_d = nc.dram_tensor("iotam_b", [128, 128], bf16, kind="ExternalInput")
    iotacf_d = nc.dram_tensor("iotac_f", [128, 1], f32, kind="ExternalInput")
    onesb_d = nc.dram_tensor("ones_b", [1, 128], bf16, kind="ExternalInput")
    onG_d = nc.dram_tensor("onG", [128, B * 128], bf16, kind="ExternalInput")
    idx_d = nc.dram_tensor("idx16", [128, TOTCOLS], i16, kind="ExternalInput")
    drc_d = nc.dram_tensor("drc", [128, TOTCH], f32, kind="ExternalInput")
    drel_d = nc.dram_tensor("drel_row", [1, TOTROW], bf16, kind="ExternalInput")
    out_d = nc.dram_tensor("pool_out", [128, Cc], f32, kind="ExternalOutput")

    stats_d = nc.dram_tensor("stats_loc", [F, 2], f32)
    stats_sh = nc.dram_tensor("stats_sh", [F, 2], f32, addr_space="Shared")
    tslice_d = nc.dram_tensor("tslice", [NPC, ROWE], bf16)
    table_sh = nc.dram_tensor("table_sh", [TROWS, ROWE], bf16, addr_space="Shared")

    seg_lo = [s * SEG for s in range(NSEG)]
    segrows = [min(SEG, TROWS - s * SEG) for s in range(NSEG)]

    with TileContext(nc) as tc, ExitStack() as ctx:
        cp = ctx.enter_context(tc.tile_pool(name="consts", bufs=1))

        def cload(name, dram, shape, dt=f32):
            t = cp.tile(shape, dt, tag=name)
            nc.sync.dma_start(out=t[:], in_=dram[:, :])
            return t

        W_t = cload("W", W_d, [F, HC])
        gamma_t = cload("gam", gamma_d, [F, 1])
        beta_t = cload("bet", beta_d, [F, 1])
        attb_t = cload("attb", attb_d, [HC, 16], bf16)
        biasm_t = cload("biasm", biasm_d, [128, Cc])
        identf_t = cload("identf", identf_d, [128, 128])
        identb_t = cload("identb", identb_d, [128, 128], bf16)
        iotamb_t = cload("iotamb", iotamb_d, [128, 128], bf16)
        iotacf_t = cload("iotacf", iotacf_d, [128, 1])
        onesb_t = cload("onesb", onesb_d, [1, 128], bf16)
        onG_t = cload("onG", onG_d, [128, B * 128], bf16)
        idx_t = cload("idx", idx_d, [128, TOTCOLS], i16)
        drc_t = cload("drc", drc_d, [128, TOTCH])
        ones_t = cp.tile([128, 1], f32, tag="ones")
        nc.vector.memset(ones_t[:], 1.0)
        scale_t = cp.tile([F, 1], f32, tag="scl")
        shift_t = cp.tile([F, 1], f32, tag="shf")
        Wp_t = cp.tile([F, HC], f32, tag="Wp")
        Wpb_t = cp.tile([F, HC], bf16, tag="Wpb")
        c0_t = cp.tile([HC, 1], f32, tag="c0")
        adw_t = cp.tile([128, B * H], bf16, tag="adw")
        xpool = tc.tile_pool(name="xp", bufs=1)
        xp = xpool.__enter__()
        xres = xp.tile([128, NT, F], f32, tag="xres")

        # ---------------- Phase A: load x slice (resident) ----------------
        XSP = 4           # node tiles per load/store DMA
        for k0 in range(0, NT, XSP):
            kk = min(XSP, NT - k0)
            nc.sync.dma_start(
                out=xres[:, k0:k0 + kk, :],
                in_=xs_d[k0 * 128:(k0 + kk) * 128, :].rearrange(
                    "(a p) f -> p a f", p=128))

        # ---------------- pass 1: BN stats + AllReduce ----------------
        with tc.tile_pool(name="p1", bufs=3) as p1, \
             tc.tile_pool(name="p1ps", bufs=1, space="PSUM") as p1ps:
            statx = p1ps.tile([F, 1], f32, tag="sx")
            statx2 = p1ps.tile([F, 1], f32, tag="sx2")
            for t in range(NT):
                sq = p1.tile([128, F], f32, tag="sq")
                nc.vector.tensor_tensor(out=sq[:], in0=xres[:, t, :],
                                        in1=xres[:, t, :], op=Alu.mult)
                nc.tensor.matmul(out=statx[:], lhsT=xres[:, t, :], rhs=ones_t[:],
                                 start=(t == 0), stop=(t == NT - 1))
                nc.tensor.matmul(out=statx2[:], lhsT=sq[:], rhs=ones_t[:],
                                 start=(t == 0), stop=(t == NT - 1))
            sloc = p1.tile([F, 2], f32, tag="sloc")
            nc.vector.tensor_copy(out=sloc[:, 0:1], in_=statx[:])
            nc.vector.tensor_copy(out=sloc[:, 1:2], in_=statx2[:])
            nc.sync.dma_start(out=stats_d[:, :], in_=sloc[:])
            nc.gpsimd.collective_compute(
                kind="AllReduce", op=Alu.add, replica_groups=groups,
                ins=[stats_d[:, :]], outs=[stats_sh[:, :]])
            tc.strict_bb_all_engine_barrier()
            sglob = p1.tile([F, 2], f32, tag="sglob")
            nc.sync.dma_start(out=sglob[:], in_=stats_sh[:, :])
            mean_t = p1.tile([F, 1], f32, tag="mean")
            tmp = p1.tile([F, 1], f32, tag="tmp")
            tmp2 = p1.tile([F, 1], f32, tag="tmp2")
            nc.vector.tensor_scalar_mul(out=mean_t[:], in0=sglob[:, 0:1], scalar1=1.0 / N)
            nc.vector.tensor_scalar_mul(out=tmp[:], in0=sglob[:, 1:2], scalar1=1.0 / N)
            nc.vector.tensor_tensor(out=tmp2[:], in0=mean_t[:], in1=mean_t[:], op=Alu.mult)
            nc.vector.tensor_tensor(out=tmp[:], in0=tmp[:], in1=tmp2[:], op=Alu.subtract)
            nc.vector.tensor_scalar_add(out=tmp[:], in0=tmp[:], scalar1=EPS)
            nc.scalar.activation(out=tmp[:], in_=tmp[:], func=Act.Sqrt)
            nc.vector.reciprocal(out=tmp2[:], in_=tmp[:])
            nc.vector.tensor_tensor(out=scale_t[:], in0=tmp2[:], in1=gamma_t[:], op=Alu.mult)
            nc.vector.tensor_tensor(out=tmp[:], in0=mean_t[:], in1=scale_t[:], op=Alu.mult)
            nc.vector.tensor_tensor(out=shift_t[:], in0=beta_t[:], in1=tmp[:], op=Alu.subtract)
            nc.vector.tensor_scalar_mul(out=Wp_t[:], in0=W_t[:], scalar1=scale_t[:, 0:1])
            nc.scalar.activation(out=Wpb_t[:], in_=Wp_t[:], func=Act.Identity)
            c0ps = p1ps.tile([HC, 1], f32, tag="c0p")
            nc.tensor.matmul(out=c0ps[:], lhsT=W_t[:], rhs=shift_t[:], start=True, stop=True)
            nc.vector.tensor_copy(out=c0_t[:], in_=c0ps[:])

        # ---------------- pass 2: node table slice ----------------
        with tc.tile_pool(name="p2", bufs=3) as p2, \
             tc.tile_pool(name="p2r", bufs=2) as p2r, \
             tc.tile_pool(name="p2ps", bufs=2, space="PSUM") as p2ps:
            row4 = None
            for t in range(NT if variant != 'a1' else 0):
                j = t % XSP
                if j == 0:
                    row4 = p2r.tile([128, XSP, ROWE], bf16, tag="row4")
                    if t < 2 * XSP:
                        # first use of each ring buffer: clear pad region once
                        nc.vector.memset(row4[:], 0.0)
                xTp = p2ps.tile([F, 128], f32, tag="xT")
                nc.tensor.transpose(out=xTp[:], in_=xres[:, t, :], identity=identf_t[:])
                xTb = p2.tile([F, 128], bf16, tag="xTb")
                nc.scalar.activation(out=xTb[:], in_=xTp[:], func=Act.Identity)
                hTp = p2ps.tile([HC, 128], f32, tag="hT")
                nc.tensor.matmul(out=hTp[:], lhsT=Wpb_t[:], rhs=xTb[:], start=True, stop=True)
                hTb = p2.tile([HC, 128], bf16, tag="hTb")
                nc.scalar.activation(out=hTb[:], in_=hTp[:], func=Act.Identity, bias=c0_t[:, 0:1])
                ap_ = p2ps.tile([128, 16], f32, tag="a")
                nc.tensor.matmul(out=ap_[:], lhsT=hTb[:], rhs=attb_t[:], start=True, stop=True)
                hpp = p2ps.tile([128, HC], bf16, tag="hp")
                nc.tensor.transpose(out=hpp[:], in_=hTb[:], identity=identb_t[:])
                nc.scalar.activation(out=row4[:, j, 0:HC], in_=hpp[:], func=Act.Identity)
                nc.vector.tensor_copy(
                    out=row4[:].bitcast(f32)[:, j, 64:72], in_=ap_[:, 0:8])
                nc.scalar.activation(out=adw_t[:, t * H:(t + 1) * H],
                                     in_=ap_[:, 8:16], func=Act.Identity)
                if j == XSP - 1 or t == NT - 1:
                    kk = j + 1
                    base = (t - j) * 128
                    nc.sync.dma_start(
                        out=tslice_d[base:base + kk * 128, :].rearrange(
                            "(a p) c -> p a c", p=128),
                        in_=row4[:, 0:kk, :])

        xpool.__exit__(None, None, None)   # free the x slice SBUF for phase B

        if variant not in ('a1', 'a2'):
            nc.gpsimd.collective_compute(
                kind="AllGather", op=Alu.bypass, replica_groups=groups,
                ins=[tslice_d[:, :]], outs=[table_sh[:, :]])
            tc.strict_bb_all_engine_barrier()

        # ---------------- Phase B ----------------
        GBUFS = 4
        gb = ctx.enter_context(tc.tile_pool(name="g", bufs=GBUFS))
        stg = ctx.enter_context(tc.tile_pool(name="stg", bufs=2))
        onep = ctx.enter_context(tc.tile_pool(name="onehots", bufs=4))
        wpool = ctx.enter_context(tc.tile_pool(name="wts", bufs=4))
        rp = ctx.enter_context(tc.tile_pool(name="rhs", bufs=4))
        pp = ctx.enter_context(tc.tile_pool(name="post", bufs=2))
        ups = ctx.enter_context(tc.tile_pool(name="ups", bufs=3, space="PSUM"))
        aps = ctx.enter_context(tc.tile_pool(name="aps", bufs=2, space="PSUM"))
        dbp = ctx.enter_context(tc.tile_pool(name="dbp", bufs=2, space="PSUM"))
        gps = ctx.enter_context(tc.tile_pool(name="gps", bufs=1, space="PSUM"))

        pool_ps = gps.tile([128, Cc], f32, tag="pool")

        cur_batch = [-1]
        cur_stage = [None]
        cur_soff = [0]
        by_block = [[] for _ in range(B)]
        for gi, (b, s, c16) in enumerate(structure):
            by_block[b].append((gi, s, c16))

        for b in range(B if variant in ('full',) else 0):
            u_ps = ups.tile([128, HC + H], f32, tag="u")
            nch_b = sum(-(-c16 // 128) for (_, _, c16) in by_block[b])
            ci = 0
            for (gi, s, c16) in by_block[b]:
                nch = -(-c16 // 128)
                L = nch * 128
                g = gb.tile([128, MAXCH, ROWE], bf16, tag="g")
                if gi < GBUFS:
                    # first use of each ring buffer: clear uninitialized SBUF so
                    # stale-NaN bit patterns can't poison pad-edge lanes; later
                    # groups only ever re-read finite gathered rows
                    nc.vector.memset(g[:], 0.0)
                nc.gpsimd.dma_gather(
                    out_ap=g[:, 0:nch, :],
                    in_ap=table_sh[seg_lo[s]:seg_lo[s] + segrows[s], :],
                    idxs_ap=idx_t[:, coloff[gi]:coloff[gi] + c16 // 16],
                    num_idxs=c16, num_idxs_reg=c16, elem_size=ROWE,
                    single_packet=False)
                if batch_of[gi] != cur_batch[0]:
                    cur_batch[0] = batch_of[gi]
                    soff, slen = stage_start[cur_batch[0]]
                    st_t = stg.tile([1, SCAP], bf16, tag="stage")
                    nc.sync.dma_start(out=st_t[0:1, 0:slen],
                                      in_=drel_d[0:1, soff:soff + slen])
                    cur_stage[0] = st_t
                    cur_soff[0] = soff
                st_t = cur_stage[0]
                roff = rowoff[gi] - cur_soff[0]

                onN = onep.tile([128, MAXCH * 128], bf16, tag="onN")
                for k in range(0, L, 512):
                    Lk = min(512, L - k)
                    dbc = dbp.tile([128, 512], f32, tag="dbc")
                    nc.tensor.matmul(out=dbc[:, 0:Lk], lhsT=onesb_t[:],
                                     rhs=st_t[0:1, roff + k:roff + k + Lk],
                                     start=True, stop=True)
                    # keep the compare off Pool: gather desc-gen queues behind
                    # it in Pool's in-order SEQ and the whole pipeline stalls
                    eng = nc.vector
                    eng.tensor_tensor(
                        out=onN[:, k:k + Lk],
                        in0=iotacf_t[:, 0:1].to_broadcast([128, Lk]),
                        in1=dbc[:, 0:Lk], op=Alu.is_equal)
                onE = onep.tile([128, MAXCH * 128], bf16, tag="onE")
                ae_ps = aps.tile([128, MAXCH * H], f32, tag="ae")
                for cch in range(nch):
                    nc.vector.tensor_scalar(
                        out=onE[:, cch * 128:(cch + 1) * 128], in0=iotamb_t[:],
                        scalar1=drc_t[:, choff[gi] + cch:choff[gi] + cch + 1],
                        scalar2=None, op0=Alu.is_equal)
                    nc.tensor.matmul(out=ae_ps[:, cch * H:(cch + 1) * H],
                                     lhsT=onN[:, cch * 128:(cch + 1) * 128],
                                     rhs=adw_t[:, b * H:(b + 1) * H],
                                     start=True, stop=True)
                egrp = wpool.tile([128, MAXCH * H], f32, tag="egrp")
                nc.vector.tensor_tensor(
                    out=egrp[:, 0:nch * H].rearrange("p (c h) -> p c h", h=H),
                    in0=g[:].bitcast(f32)[:, 0:nch, 64:72],
                    in1=ae_ps[:, 0:nch * H].rearrange("p (c h) -> p c h", h=H),
                    op=Alu.add)
                t1 = wpool.tile([128, MAXCH * H], f32, tag="t1")
                nc.vector.scalar_tensor_tensor(
                    out=t1[:, 0:nch * H], in0=egrp[:, 0:nch * H], scalar=NEG_SLOPE,
                    in1=egrp[:, 0:nch * H], op0=Alu.mult, op1=Alu.max)
                # expand exp(logit) to all 16 channels on ACT (same Exp table,
                # broadcast input) so the weight-multiply runs packed-2x on DVE
                wtx = wpool.tile([128, MAXCH, H, Cc], bf16, tag="wtx")
                nc.scalar.activation(
                    out=wtx[:, 0:nch, :, :],
                    in_=t1[:, 0:nch * H].rearrange("p (c h one) -> p c h one", h=H,
                                                   one=1).to_broadcast([128, nch, H, Cc]),
                    func=Act.Exp)
                rhs_t = rp.tile([128, MAXCH, HC + H], bf16, tag="rhs")
                nc.vector.tensor_tensor(
                    out=rhs_t[:, 0:nch, 0:HC].rearrange("p c (h c2) -> p c h c2", h=H),
                    in0=g[:, 0:nch, 0:HC].rearrange("p c (h c2) -> p c h c2", h=H),
                    in1=wtx[:, 0:nch, :, :],
                    op=Alu.mult)
                nc.vector.tensor_copy(out=rhs_t[:, 0:nch, HC:HC + H],
                                      in_=wtx[:, 0:nch, :, 0].rearrange("p c h -> p (c h)"))
                for cch in range(nch):
                    nc.tensor.matmul(out=u_ps[:], lhsT=onE[:, cch * 128:(cch + 1) * 128],
                                     rhs=rhs_t[:, cch, :],
                                     start=(ci == 0), stop=(ci == nch_b - 1))
                    ci += 1
            # ---- postprocess block ----
            s_sb = pp.tile([128, H], f32, tag="s")
            nc.vector.tensor_scalar_add(out=s_sb[:], in0=u_ps[:, HC:HC + H], scalar1=1e-30)
            rs = pp.tile([128, H], f32, tag="rs")
            nc.vector.reciprocal(out=rs[:], in_=s_sb[:])
            prod = pp.tile([128, HC], f32, tag="prod")
            nc.vector.tensor_tensor(
                out=prod[:].rearrange("p (h c2) -> p h c2", h=H),
                in0=u_ps[:, 0:HC].rearrange("p (h c2) -> p h c2", h=H),
                in1=rs[:].rearrange("p (h one) -> p h one", h=H
                                    ).to_broadcast([128, H, Cc]),
                op=Alu.mult)
            o16 = pp.tile([128, Cc], f32, tag="o16")
            nc.vector.tensor_reduce(out=o16[:], in_=prod[:].rearrange("p (h c2) -> p c2 h", h=H),
                                    axis=mybir.AxisListType.X, op=Alu.add)
            o16b = pp.tile([128, Cc], f32, tag="o16b")
            nc.vector.scalar_tensor_tensor(out=o16b[:], in0=o16[:], scalar=1.0 / H,
                                           in1=biasm_t[:], op0=Alu.mult, op1=Alu.add)
            m0 = pp.tile([128, Cc], f32, tag="m0")
            nc.vector.tensor_scalar(out=m0[:], in0=o16b[:], scalar1=0.0, scalar2=None,
                                    op0=Alu.min)
            em = pp.tile([128, Cc], f32, tag="em")
            nc.scalar.activation(out=em[:], in_=m0[:], func=Act.Exp)
            r0 = pp.tile([128, Cc], f32, tag="r0")
            nc.vector.scalar_tensor_tensor(out=r0[:], in0=m0[:], scalar=-1.0,
                                           in1=o16b[:], op0=Alu.mult, op1=Alu.add)
            onode = pp.tile([128, Cc], bf16, tag="onode")
            nc.vector.scalar_tensor_tensor(out=onode[:], in0=em[:], scalar=-1.0,
                                           in1=r0[:], op0=Alu.add, op1=Alu.add)
            nc.tensor.matmul(out=pool_ps[:], lhsT=onG_t[:, b * 128:(b + 1) * 128],
                             rhs=onode[:], start=(b == 0), stop=(b == B - 1))

        if variant in ('full',):
            outp_t = pp.tile([128, Cc], f32, tag="out")
            nc.vector.tensor_copy(out=outp_t[:], in_=pool_ps[:])
            nc.sync.dma_start(out=out_d[:, :], in_=outp_t[:])
        else:
            outp_t = pp.tile([128, Cc], f32, tag="out")
            nc.vector.memset(outp_t[:], 0.0)
            nc.sync.dma_start(out=out_d[:, :], in_=outp_t[:])

    nc.compile()
    return nc


def _np_f32(a):
    return np.ascontiguousarray(np.asarray(a), dtype=np.float32)


def make_in_maps(cfg, prep, inputs):
    c = cfg
    F, H, Cc, HC, NC, NPC, B, N = (c["F"], c["H"], c["C"], c["HC"], c["NCORES"],
                                   c["NPC"], c["B"], c["N"])
    x = _np_f32(inputs["x"])
    W = _np_f32(inputs["W"])
    gamma = _np_f32(inputs["bn_gamma"]).reshape(F, 1)
    beta = _np_f32(inputs["bn_beta"]).reshape(F, 1)
    att_src = _np_f32(inputs["att_src"])
    att_dst = _np_f32(inputs["att_dst"])
    bias = _np_f32(inputs["bias"]).reshape(1, Cc)

    def bf(a):
        import jax.numpy as jnp
        return np.asarray(jnp.asarray(a, dtype=jnp.bfloat16))

    attboth = np.zeros((HC, 16), np.float32)
    for h in range(H):
        attboth[h * Cc:(h + 1) * Cc, h] = att_src[h]
        attboth[h * Cc:(h + 1) * Cc, 8 + h] = att_dst[h]

    xpad = np.zeros((NPC * NC, F), np.float32)
    xpad[:N] = x

    iotam = np.tile(np.arange(128, dtype=np.float32), (128, 1))
    shared = dict(
        W=W, gamma=gamma, beta=beta,
        attboth=bf(attboth),
        bias_mat=np.tile(bias, (128, 1)),
        ident_f=np.eye(128, dtype=np.float32),
        ident_b=bf(np.eye(128)),
        iotam_b=bf(iotam),
        iotac_f=np.arange(128, dtype=np.float32).reshape(128, 1),
        ones_b=bf(np.ones((1, 128))),
    )
    in_maps = []
    for m in range(NC):
        im = dict(shared)
        im["xs"] = xpad[m * NPC:(m + 1) * NPC]
        im["idx16"] = prep["idx16"][m]
        im["drc"] = prep["drc"][m]
        im["drel_row"] = bf(prep["drel_row"][m])
        im["onG"] = bf(prep["onG"][m])
        in_maps.append(im)
    return in_maps


def unshard(cfg, prep, results):
    c = cfg
    G, Cc, NC = c["G"], c["C"], c["NCORES"]
    batchcnt = prep["graph_counts"]
    out = np.zeros((G, Cc), np.float64)
    for m in range(NC):
        pool_m = results[m]["pool_out"]
        g0 = int(prep["g0"][m])
        hi = min(128, G - g0)
        out[g0:g0 + hi] += pool_m[:hi]
    out = out / np.maximum(batchcnt, 1.0)[:, None]
    return out.astype(np.float32)


_CACHE = {}
LAST = {}


def kernel(**inputs):
    from concourse.bass_utils import run_bass_kernel_spmd

    cfg = _derive(_default_cfg())
    batch = np.asarray(inputs["batch"]).astype(np.int64)
    prep = host_prep(cfg, inputs["edge_index"], batch)
    prep["graph_counts"] = np.bincount(batch, minlength=cfg["G"]).astype(np.float64)
    key = "full"
    if key not in _CACHE:
        _CACHE[key] = build_nc(cfg, prep, cfg["NCORES"])
    nc = _CACHE[key]
    in_maps = make_in_maps(cfg, prep, inputs)
    res = run_bass_kernel_spmd(nc, in_maps, list(range(cfg["NCORES"])))
    LAST["res"] = res
    return unshard(cfg, prep, res.results)
